# revision 12
# baseline (speedup 1.0000x reference)
"""Additive (Bahdanau) attention on 4 of 8 TRN2 NeuronCores.

Problem shapes: B=4, Q=512, K=1024, Dq=Dk=Dv=512, H=128.

Sharding: one batch per core on a 4-core mesh (cores 4-7 idle). The
metric this kernel is tuned for is the MARGINAL WALL-CLOCK PER DISPATCH
through the axon tunnel, and that cost scales with mesh size (~100 us
fixed + ~15-30 us per core: measured sustained slopes for a trivial
NEFF are 99/92/116/233 us at 1/2/4/8 cores), while the on-device time
scales down with more cores. 8 cores put the device at ~72 us but
dispatch at ~233 us; 4 cores put dispatch at ~116 us and the device at
~95-115 us (each core shares ALL key-side work -- k features, k trig,
values cast -- across its two query halves, so 4-core device time is
well under 2x the 8-core time). max(dispatch, device) is minimized at
4 cores.

Algorithm (sine decomposition of additive attention):

tanh(a+b) is separable through the angle-addition identity. Fit
tanh(x) ~ sum_r c_r sin(w_r x) (weighted least squares, R=8, wmax=4.5,
core max err ~2e-3 over the feature-sum range), then

  scores[q,k] = sum_h w_h tanh(qf_hq + kf_hk)
             = sum_r [ (c_r w_h sin(w_r qf)) . cos(w_r kf)
                     + (c_r w_h cos(w_r qf)) . sin(w_r kf) ]   (contract h)

i.e. 2R=16 accumulating 128-contraction matmuls on the tensor engine
instead of 268M scalar-engine tanh evaluations (~300 us/core direct).
The ACT Sin table is only accurate on [-pi, pi], so arguments are
range-reduced exactly:

  t   = x * (w/2pi)                 (DVE tensor_scalar; ACT Copy q-side)
  a_s = fl(t + 1.5*2^23)            (ACT Copy with float bias: the fp32
                                     store rounds t to the nearest
                                     integer; bit-identical to the DVE
                                     tensor_scalar version)
  e_s = (a_s - 1.5*2^23) - t        (DVE scalar_tensor_tensor; both
                                     terms exact) = round(t) - t
  sin(w x) = sin(-2pi * e_s)        (ACT Sin, scale = -2pi)
  e_c  = wrap(e_s - 1/4)            (one DVE add_range_wrap into
                                     [-1/2, 1/2]; -2pi*e_c = -2pi*e_s
                                     + pi/2 (mod 2pi))
  cos(w x) = sin(-2pi * e_c)        (same ACT Sin, no bias -- the whole
                                     second reduction chain collapses
                                     into one custom-DVE op)

IMPORTANT: no GPSIMD (Q7) instructions anywhere -- each dispatch of a
NEFF containing GPSIMD ops costs ~300-400 us of extra per-dispatch
host/runtime overhead under axon, dwarfing the on-device time. The
trig chains are balanced across DVE and ACT only; k-side ops are fused
1024 wide. sin/cos tiles are bf16 (the c_r*w_h weighting is folded
into the q-side, keeping per-term magnitudes small). Inputs arrive as
ONE packed DRAM parameter (fewer PJRT buffer binds per dispatch).

Score accumulation note: matmul start=True clears the has_written bits
of the whole PSUM bank, so interleaved accumulation groups sharing a
bank cannot use it. A dummy full-bank start=True matmul writes zeros
and sets every bit; the real matmuls then accumulate with start=False
in r-streaming order. Scores for the two query halves go through the
same 4-bank PSUM region sequentially (8 banks total: 4 scores + 2x2
output/denominator); the tile framework's WAR tracking makes half B's
zeroing wait for half A's exp reads, and the PE fills that bubble with
half A's attn@V matmuls.

Softmax needs no max-subtraction (scores are O(1) by construction: w_v
has variance 1/H). exp runs once over the transposed scores [k, q] in
PSUM so the attn tile is directly the stationary operand of the attn@V
matmuls; the softmax denominator comes from one extra accumulating
matmul against a ones vector, followed by a DVE reciprocal and a
per-partition rescale of the output.

Dispatch-path notes (the dominant cost): the runner compiles under
bass2jax._fast_dispatch_active(True) so _bass_exec_p declares no
effect and calls take jax's C++ pjit fast path (the Python
effects/token path costs ~0.3-0.7 ms/call more); it returns the raw
Compiled rather than the FastDispatchCompiled wrapper (whose per-call
Python re-registration of every output shard costs ~0.1-0.2 ms); and
it passes no zero-filled output operands (on the exec lowering path
the NEFF binds only BIR ExternalInputs, and this kernel DMA-writes
every element of out). partition_id is disabled (unused input).
"""


import os
import numpy as np

import concourse.bass as bass
import concourse.mybir as mybir
import concourse.tile as tile
from concourse import bacc
from concourse.bass_utils import run_bass_kernel_spmd
from concourse.masks import make_identity

B, Q, K, D, H = 4, 512, 1024, 512, 128
N_CORES = 4
QSH = Q                         # 512 query rows per core (one batch/core)
QH = 256                        # query rows per PSUM score pass
NDC = D // 128                  # 4 contraction chunks
NKC = K // 128                  # 8 key chunks
NQB = QSH // 128                # 4 query blocks per core

F32 = mybir.dt.float32
BF16 = mybir.dt.bfloat16
EXP = mybir.ActivationFunctionType.Exp
SIN = mybir.ActivationFunctionType.Sin
COPY = mybir.ActivationFunctionType.Copy
TS = mybir.AluOpType

MAGIC = 12582912.0              # 1.5 * 2**23: fp32 add forces round-to-int
TWO_PI = float(2.0 * np.pi)

LAST_EXEC_NS = None
_NC_CACHE = {}


R_SINE = 8
WMAX = 4.5

def _fit_sine(R=R_SINE, wmax=WMAX, L=8.5, sigma=1.7):
    """Least-squares fit tanh(x) ~ sum_r c_r sin(w_r x) on [-L, L]."""
    ws = np.linspace(wmax / R * 0.5, wmax, R)
    xs = np.linspace(-L, L, 4001)
    wt = np.exp(-xs ** 2 / (2 * sigma ** 2)) + 1e-3
    A = np.sin(np.outer(xs, ws))
    Wt = np.sqrt(wt)[:, None]
    c, *_ = np.linalg.lstsq(A * Wt, np.tanh(xs) * Wt[:, 0], rcond=None)
    return [float(w) for w in ws], [float(v) for v in c]


# Packed single-input layout (one NEFF parameter instead of six: fewer PJRT
# buffer binds per dispatch, which dominates the per-call overhead under axon).
OFF_Q = 0
OFF_K = OFF_Q + QSH * D          # 262144
OFF_V = OFF_K + K * D            # 786432
OFF_WQ = OFF_V + K * D           # 1310720
OFF_WK = OFF_WQ + D * H          # 1376256
OFF_WV = OFF_WK + D * H          # 1441792
NPACK = OFF_WV + H               # 1441920


def _declare_io(nc):
    xin = nc.declare_dram_parameter("xin", [NPACK], F32, isOutput=False)
    out_ext = nc.declare_dram_parameter("out", [QSH, D], F32, isOutput=True)
    q_ext = xin[OFF_Q:OFF_K].rearrange("(q d) -> q d", d=D)
    k_ext = xin[OFF_K:OFF_V].rearrange("(k d) -> k d", d=D)
    v_ext = xin[OFF_V:OFF_WQ].rearrange("(k d) -> k d", d=D)
    wq_ext = xin[OFF_WQ:OFF_WK].rearrange("(d h) -> d h", h=H)
    wk_ext = xin[OFF_WK:OFF_WV].rearrange("(d h) -> d h", h=H)
    wv_ext = xin[OFF_WV:NPACK].rearrange("(h o) -> h o", o=1)
    return q_ext, k_ext, v_ext, wq_ext, wk_ext, wv_ext, out_ext


def _preamble(nc, tc, const, work, feat, q_ext, k_ext, v_ext, wq_ext, wk_ext,
              wv_ext):
    """DMA + PE-transpose inputs, feature matmuls, values cast.

    Returns (qf_sb [H, QSH] f32, kf_sb [H, K] f32, v_b [128, NKC, D] bf16,
    wv_f [H,1] f32, ones_b [128,1] bf16)."""
    ident = const.tile([128, 128], F32)
    make_identity(nc, ident)

    wq_t = const.tile([128, NDC, H], F32)
    wk_t = const.tile([128, NDC, H], F32)
    nc.sync.dma_start(out=wq_t, in_=wq_ext.rearrange("(c p) h -> p c h", p=128))
    nc.sync.dma_start(out=wk_t, in_=wk_ext.rearrange("(c p) h -> p c h", p=128))

    wv_f = const.tile([H, 1], F32)
    nc.sync.dma_start(out=wv_f, in_=wv_ext[:])
    ones_b = const.tile([128, 1], BF16)
    nc.vector.memset(ones_b, 1.0)

    qT = feat.tile([128, NDC, QSH], F32)
    kT = feat.tile([128, NDC, K], F32)
    qf_sb = feat.tile([H, QSH], F32)
    kf_sb = feat.tile([H, K], F32)
    with tc.tile_pool(name="kwork", bufs=8) as kwork, \
         tc.tile_pool(name="pre_ps", bufs=2, space="PSUM") as pre_ps:
        for t in range(QSH // 128):
            q_in = work.tile([128, D], F32, tag="qin")
            (nc.sync, nc.scalar)[t % 2].dma_start(
                out=q_in, in_=q_ext[t * 128:(t + 1) * 128, :])
            for dc in range(NDC):
                tp = pre_ps.tile([128, 128], F32, tag="tps")
                nc.tensor.transpose(tp, q_in[:, dc * 128:(dc + 1) * 128], ident)
                dst = qT[:, dc, t * 128:(t + 1) * 128]
                (nc.vector.tensor_copy(dst, tp) if dc % 2 == 0
                 else nc.scalar.copy(dst, tp))

        for t in range(K // 128):
            k_in = kwork.tile([128, D], F32, tag="kin")
            dma_eng = (nc.sync, nc.scalar)[t % 2]
            dma_eng.dma_start(out=k_in, in_=k_ext[t * 128:(t + 1) * 128, :])
            for dc in range(NDC):
                tp = pre_ps.tile([128, 128], F32, tag="tps")
                nc.tensor.transpose(tp, k_in[:, dc * 128:(dc + 1) * 128], ident)
                dst = kT[:, dc, t * 128:(t + 1) * 128]
                (nc.vector.tensor_copy(dst, tp) if dc % 2 == 0
                 else nc.scalar.copy(dst, tp))

        qf_ps = pre_ps.tile([H, QSH], F32, tag="fps")
        for dc in range(NDC):
            nc.tensor.matmul(qf_ps, wq_t[:, dc, :], qT[:, dc, :],
                             start=(dc == 0), stop=(dc == NDC - 1))
        nc.vector.tensor_copy(qf_sb, qf_ps)

        for hf in range(2):
            kf_ps = pre_ps.tile([H, 512], F32, tag="fps")
            for dc in range(NDC):
                nc.tensor.matmul(kf_ps, wk_t[:, dc, :],
                                 kT[:, dc, hf * 512:(hf + 1) * 512],
                                 start=(dc == 0), stop=(dc == NDC - 1))
            dst = kf_sb[:, hf * 512:(hf + 1) * 512]
            (nc.vector.tensor_copy(dst, kf_ps) if hf == 0
             else nc.scalar.copy(dst, kf_ps))

    v_b = feat.tile([128, NKC, D], BF16)
    tc.tile_set_cur_wait(0.05)   # keep values off the keys->kf critical path
    for kc in range(NKC):
        v_in = work.tile([128, D], F32, tag="vin")
        (nc.sync, nc.scalar)[kc % 2].dma_start(
            out=v_in, in_=v_ext[kc * 128:(kc + 1) * 128, :])
        (nc.vector.tensor_copy if kc % 2 else nc.scalar.copy)(
            v_b[:, kc, :], v_in)
    tc.tile_set_cur_wait(0)

    return qf_sb, kf_sb, v_b, wv_f, ones_b


def _build_sine():
    ws, cs = _fit_sine()
    R = len(ws)
    nc = bacc.Bacc(enable_partition_id=False)
    q_ext, k_ext, v_ext, wq_ext, wk_ext, wv_ext, out_ext = _declare_io(nc)

    with tile.TileContext(nc) as tc:
        with tc.tile_pool(name="const", bufs=1) as const, \
             tc.tile_pool(name="work", bufs=3) as work, \
             tc.tile_pool(name="feat", bufs=1) as feat, \
             tc.tile_pool(name="trig", bufs=2) as trig, \
             tc.tile_pool(name="oloop", bufs=2) as oloop:

            qf_sb, kf_sb, v_b, wv_f, ones_b = _preamble(
                nc, tc, const, work, feat, q_ext, k_ext, v_ext,
                wq_ext, wk_ext, wv_ext)

            # per-r q-side coefficient vectors: wc[:, r] = c_r * w_v
            wc = const.tile([H, R], F32)
            for r in range(R):
                nc.vector.tensor_scalar_mul(wc[:, r:r + 1], wv_f, float(cs[r]))

            KS = feat.tile([H, R, K], BF16)
            KC = feat.tile([H, R, K], BF16)
            QS = feat.tile([H, R, QSH], BF16)
            QC = feat.tile([H, R, QSH], BF16)


            def trig_chain(x_sl, width, out_s, out_c, scale_col):
                """out_s = sin(w*x), out_c = cos(w*x). Exact range reduction:
                t = x*(w/2pi); a = fl(t + 1.5*2^23) (fp32 store rounds t to
                the nearest integer); e_s = (a - MAGIC) - t = round(t) - t
                (both subtractions exact); sin(w*x) = sin(-2pi*e_s). The cos
                argument reuses e_s through one custom-DVE wrap:
                e_c = wrap(e_s - 1/4) into [-1/2, 1/2], so -2pi*e_c =
                -2pi*e_s + pi/2 (mod 2pi) and sin(-2pi*e_c) = cos(w*x) with
                no activation bias and no second reduction chain. All ops
                run on DVE/ACT only: GPSIMD (Q7) instructions add ~300us of
                per-dispatch host overhead under axon."""
                t_t = trig.tile([H, width], F32, tag=f"t{width}")
                a_s = trig.tile([H, width], F32, tag=f"as{width}")
                if scale_col is None:
                    # k-side: t on DVE, the MAGIC round on ACT (Copy's f32
                    # store rounds t+MAGIC to the nearest integer just like
                    # the DVE tensor_scalar store does).
                    nc.vector.tensor_scalar(t_t, x_sl, w2p, None, TS.mult)
                    nc.scalar.activation(out=a_s, in_=t_t, func=COPY,
                                         bias=MAGIC)
                else:
                    # q-side (2x narrower): both on ACT to unload DVE.
                    nc.scalar.activation(out=t_t, in_=x_sl, func=COPY,
                                         scale=w2p)
                    nc.scalar.activation(out=a_s, in_=t_t, func=COPY,
                                         bias=MAGIC)
                e_s = trig.tile([H, width], F32, tag=f"es{width}")
                nc.vector.scalar_tensor_tensor(e_s, a_s, MAGIC, t_t,
                                               TS.subtract, TS.subtract)
                e_c = trig.tile([H, width], F32, tag=f"ec{width}")
                nc.vector.add_range_wrap(e_c, e_s, -0.25, 0.5, 1.0)
                if scale_col is None:
                    nc.scalar.activation(out=out_s, in_=e_s, func=SIN,
                                         scale=-TWO_PI)
                    nc.scalar.activation(out=out_c, in_=e_c, func=SIN,
                                         scale=-TWO_PI)
                else:
                    s_t = trig.tile([H, width], F32, tag=f"ss{width}")
                    nc.scalar.activation(out=s_t, in_=e_s, func=SIN,
                                         scale=-TWO_PI)
                    nc.vector.tensor_scalar_mul(out_s, s_t, scale_col)
                    c_t = trig.tile([H, width], F32, tag=f"sc{width}")
                    nc.scalar.activation(out=c_t, in_=e_c, func=SIN,
                                         scale=-TWO_PI)
                    nc.vector.tensor_scalar_mul(out_c, c_t, scale_col)

            for r in range(R):
                w2p = float(ws[r] / TWO_PI)
                wcol = wc[:, r:r + 1]
                trig_chain(kf_sb, K, KS[:, r, :], KC[:, r, :], None)
                trig_chain(qf_sb, QSH, QS[:, r, :], QC[:, r, :], wcol)

            zeros_b = const.tile([128, 512], BF16)
            nc.vector.memset(zeros_b, 0.0)

            with tc.tile_pool(name="ps", bufs=2, space="PSUM") as ps, \
                 tc.tile_pool(name="ps4", bufs=1, space="PSUM") as ps4:
                for qh in range(QSH // QH):
                    qsl = slice(qh * QH, (qh + 1) * QH)
                    # start=True clears has_written for the WHOLE bank, so
                    # interleaved accumulation groups sharing a bank must
                    # not use it. Instead: one full-bank dummy start=True
                    # matmul writes zeros + sets every has_written bit; all
                    # real matmuls then accumulate with start=False in
                    # r-streaming order. ps4 has bufs=1: half B reuses the
                    # same 4 banks, WAR-serialized behind half A's exp.
                    sc_ps = ps4.tile([128, NKC, QH], F32, tag="sc")
                    for bank in range(NKC * QH // 512):
                        region = sc_ps[:, 2 * bank:2 * bank + 2, :]
                        nc.tensor.matmul(region, zeros_b[:, :128], zeros_b,
                                         start=True, stop=False,
                                         skip_group_check=True)
                    for r in range(R):
                        for kc in range(NKC):
                            ksl = slice(kc * 128, (kc + 1) * 128)
                            nc.tensor.matmul(sc_ps[:, kc, :],
                                             KC[:, r, ksl], QS[:, r, qsl],
                                             start=False, stop=False,
                                             skip_group_check=True)
                            nc.tensor.matmul(sc_ps[:, kc, :],
                                             KS[:, r, ksl], QC[:, r, qsl],
                                             start=False, stop=(r == R - 1),
                                             skip_group_check=True)
                    for qb in range(QH // 128):
                        attnT = oloop.tile([128, NKC, 128], BF16, tag="attnT")
                        nc.scalar.activation(
                            out=attnT, in_=sc_ps[:, :, qb * 128:(qb + 1) * 128],
                            func=EXP)
                        o_ps = ps.tile([128, D], F32, tag="ops")
                        d_ps = ps.tile([128, 1], F32, tag="dps")
                        for kc in range(NKC):
                            nc.tensor.matmul(o_ps, attnT[:, kc, :],
                                             v_b[:, kc, :],
                                             start=(kc == 0),
                                             stop=(kc == NKC - 1))
                            nc.tensor.matmul(d_ps, attnT[:, kc, :], ones_b,
                                             start=(kc == 0),
                                             stop=(kc == NKC - 1))
                        recip = oloop.tile([128, 1], F32, tag="recip")
                        nc.vector.reciprocal(recip, d_ps)
                        o_sb = oloop.tile([128, D], F32, tag="osb")
                        nc.vector.tensor_scalar_mul(o_sb, o_ps, recip)
                        row = qh * QH + qb * 128
                        (nc.sync, nc.scalar)[qb % 2].dma_start(
                            out=out_ext[row:row + 128, :], in_=o_sb)
    nc.compile()
    return nc


def _get_nc():
    if "sine" not in _NC_CACHE:
        _NC_CACHE["sine"] = _build_sine()
    return _NC_CACHE["sine"]


def make_in_maps(queries, keys, values, W_q, W_k, w_v):
    queries = np.asarray(queries, dtype=np.float32)
    keys = np.asarray(keys, dtype=np.float32)
    values = np.asarray(values, dtype=np.float32)
    W_q = np.asarray(W_q, dtype=np.float32).ravel()
    W_k = np.asarray(W_k, dtype=np.float32).ravel()
    w_v = np.asarray(w_v, dtype=np.float32).ravel()
    in_maps = []
    for c in range(N_CORES):
        buf = np.empty(NPACK, np.float32)
        buf[OFF_Q:OFF_K] = queries[c].ravel()
        buf[OFF_K:OFF_V] = keys[c].ravel()
        buf[OFF_V:OFF_WQ] = values[c].ravel()
        buf[OFF_WQ:OFF_WK] = W_q
        buf[OFF_WK:OFF_WV] = W_k
        buf[OFF_WV:NPACK] = w_v
        in_maps.append({"xin": buf})
    return in_maps


_RUNNER_CACHE = {}


def _get_runner(nc):
    """Persistent compiled shard_map runner for nc (compiled once/process).

    Two dispatch-path choices matter for the marginal per-call cost under
    axon (the per-dispatch host overhead dominates on-device time):

    * compile under bass2jax._fast_dispatch_active(True): _bass_exec_p then
      declares no effect, so calls take jax's C++ pjit fast path instead of
      the Python effects/token dispatch (~0.3-0.7 ms/call cheaper).
    * return the raw Compiled, NOT FastDispatchCompiled: the safety-net
      wrapper re-registers every output shard in runtime_tokens on every
      call (a Python loop over the shards, ~0.1-0.2 ms/call). kernel()
      reads its outputs immediately, so device errors surface regardless.
    * no zero-filled output operands: on the exec lowering path the NEFF
      binds only BIR ExternalInputs (the "out" zeros param has no NEFF
      tensor and is ignored), and this kernel DMA-writes every element of
      out, so PJRT's uninitialized result allocation is fine. Dropping
      them saves one buffer bind per core per call.
    """
    if id(nc) in _RUNNER_CACHE:
        return _RUNNER_CACHE[id(nc)]
    import jax
    from jax.sharding import Mesh, NamedSharding, PartitionSpec
    from jax.experimental.shard_map import shard_map
    from concourse import bass2jax

    bass2jax.install_neuronx_cc_hook()
    partition_name = (nc.partition_id_tensor.name
                      if nc.partition_id_tensor else None)
    in_names, in_shapes, out_names, out_avals = [], [], [], []
    for alloc in nc.m.functions[0].allocations:
        if not isinstance(alloc, mybir.MemoryLocationSet):
            continue
        name = alloc.memorylocations[0].name
        if alloc.kind == "ExternalInput":
            if name != partition_name:
                in_names.append(name)
                in_shapes.append(
                    (tuple(alloc.tensor_shape), mybir.dt.np(alloc.dtype)))
        elif alloc.kind == "ExternalOutput":
            out_names.append(name)
            shape = tuple(alloc.tensor_shape)
            dtype = mybir.dt.np(alloc.dtype)
            out_avals.append(jax.core.ShapedArray(shape, dtype))
    all_in_names = list(in_names)
    if partition_name is not None:
        all_in_names.append(partition_name)

    def _body(*args):
        operands = list(args)
        if partition_name is not None:
            operands.append(bass2jax.partition_id_tensor())
        outs = bass2jax._bass_exec_p.bind(
            *operands,
            out_avals=tuple(out_avals),
            in_names=tuple(all_in_names),
            out_names=tuple(out_names),
            lowering_input_output_aliases=(),
            sim_require_finite=True,
            sim_require_nnan=True,
            nc=nc,
        )
        return tuple(outs)

    devices = jax.devices()[:N_CORES]
    mesh = Mesh(np.asarray(devices), ("core",))
    nio = len(in_names)
    sharding = NamedSharding(mesh, PartitionSpec("core"))
    fast_ctx = getattr(bass2jax, "_fast_dispatch_active", None)
    import contextlib
    with (fast_ctx(True) if fast_ctx is not None
          else contextlib.nullcontext()):
        f = jax.jit(
            shard_map(_body, mesh=mesh,
                      in_specs=(PartitionSpec("core"),) * nio,
                      out_specs=(PartitionSpec("core"),) * len(out_names),
                      check_rep=False),
            keep_unused=True,
        )
        dummy_in = [
            jax.ShapeDtypeStruct((N_CORES * shape[0], *shape[1:]), dtype,
                                 sharding=sharding)
            for shape, dtype in in_shapes
        ]
        fc = f.lower(*dummy_in).compile()
    runner = (fc, in_names, out_names, out_avals, sharding)
    _RUNNER_CACHE[id(nc)] = runner
    return runner


def kernel(queries, keys, values, W_q, W_k, w_v):
    import jax
    nc = _get_nc()
    in_maps = make_in_maps(queries, keys, values, W_q, W_k, w_v)
    try:
        fc, in_names, out_names, out_avals, sharding = _get_runner(nc)
        concat_in = [
            np.concatenate([in_maps[c][name] for c in range(N_CORES)], axis=0)
            for name in in_names
        ]
        args = [jax.device_put(a, sharding) for a in concat_in]
        out_arrs = fc(*args)
        results = [
            {name: np.asarray(out_arrs[i]).reshape(
                N_CORES, *out_avals[i].shape)[c]
             for i, name in enumerate(out_names)}
            for c in range(N_CORES)
        ]
    except Exception:
        res = run_bass_kernel_spmd(nc, in_maps, core_ids=list(range(N_CORES)))
        results = res.results

    out = np.empty((B, Q, D), dtype=np.float32)
    for c in range(N_CORES):
        out[c] = results[c]["out"]
    return out
